# revision 12
# baseline (speedup 1.0000x reference)
"""Trainium2 Bass kernel for the AnaphoricityScorer problem.

Data-parallel over the batch (mention) dimension across 8 NeuronCores.
Per core: 64 mentions x 50 antecedents = 3200 pair rows, r = ant*64 + m.

pair = [a, b, a*b, pw] @ W1 with the a-term folded into a fused k-tile
(T_a' = mentions @ W1a + b1 injected through a 0/1 selection matrix S).
The b and a*b terms run as fp8(e4m3) DoubleRow matmuls (2 k-tiles per
instruction, 2x bf16 throughput); to condition the fp8 weights, the
whole pre-activation is scaled by 64 (W1*64, b1*64 exact power-of-2
scalings) and undone through W2/64 - valid because LeakyReLU is
positively homogeneous. Gathers, transposes and the fused tile run in
bf16. Gathered rows transpose on the TensorEngine (bf16, 1 cycle/row)
into PSUM; per 128-row tile one Scalar-engine cast makes the fp8 b^T
slabs and one DVE multiply makes the fp8 (a*b)^T slabs, laid out
[128, 2, N] for DoubleRow consumption.
"""

import os
import sys
from contextlib import ExitStack

import numpy as np
import ml_dtypes

for _p in ("/opt/trn_rl_repo",):
    if _p not in sys.path and os.path.isdir(_p):
        sys.path.insert(0, _p)

from concourse import bass, mybir  # noqa: E402
import concourse.tile as tile  # noqa: E402
from concourse.masks import make_identity  # noqa: E402
from concourse.bass_utils import run_bass_kernel_spmd  # noqa: E402

NM, BATCH, A, E, PW, HID, NCORES = 2000, 512, 50, 1024, 64, 1024, 8
BS = BATCH // NCORES
R = A * BS
ALPHA, EPSILON = 0.01, 1e-07
SC = 64.0
F32 = mybir.dt.float32
BF16 = mybir.dt.bfloat16
FP8 = mybir.dt.float8e4
I32 = mybir.dt.int32
DRM = mybir.MatmulPerfMode.DoubleRow
KE, NT, NG = E // 128, HID // 128, E // 256

_CH = [512, 512, 512, 512, 512, 512, 128]
_RCS = [0, 512, 1024, 1536, 2048, 2560, 3072]
NCHUNK = len(_CH)

BF_NP = ml_dtypes.bfloat16
F8_NP = ml_dtypes.float8_e4m3


def _redistribute_waits(nc, helper_sems, limit=1):
    """Enforce <=1 sync wait per instruction (walrus limit on this build).

    Compute-engine instructions execute in-order on their engine stream, so
    excess waits hoist into single-wait InstEventSemaphore instructions
    spliced just before them. DMACopy instructions execute from concurrent
    DGE queue programs, so an engine-stream EventSem does NOT gate them:
    their waits are bridged through a per-engine helper semaphore - the
    EventSems consume the original waits on the engine stream and increment
    the helper; the DMA's single wait slot watches the helper's cumulative
    count. Helpers are decremented back to zero at the end so repeated
    executions of the loaded NEFF stay correct.
    """
    counter = [0]
    counts = {e: 0 for e in helper_sems}
    last_blk = None

    def mk_ev(engine, wait=None, update=None):
        ev = mybir.InstEventSemaphore(
            name=f"hoistw-{counter[0]}", ins=[], outs=[]
        )
        counter[0] += 1
        ev.engine = engine
        ev.sync_info = mybir.SyncInfo(
            on_wait=[wait] if wait else [], on_update=[update] if update else []
        )
        return ev

    for f in nc.m.functions:
        for blk in f.blocks:
            il = blk.instructions
            if il:
                last_blk = blk
            new_il = []
            changed = False
            for inst in il:
                si = inst.sync_info
                waits = list(si.on_wait) if si is not None else []
                if isinstance(inst, mybir.InstDMACopy) and len(waits) > limit:
                    h = helper_sems[inst.engine]
                    for i, w in enumerate(waits):
                        upd = None
                        if i == len(waits) - 1:
                            upd = mybir.SyncUpdate(
                                sync_type="semaphore",
                                id=h.num,
                                ant_name=h.name,
                                update_mode="sem-inc",
                                update_value=1,
                            )
                        new_il.append(mk_ev(inst.engine, w, upd))
                    counts[inst.engine] += 1
                    si.on_wait = [
                        mybir.SyncWait(
                            sync_type="semaphore",
                            id=h.num,
                            ant_name=h.name,
                            wait_mode="sem-ge-imm",
                            wait_value=counts[inst.engine],
                        )
                    ]
                    changed = True
                elif len(waits) > limit:
                    for w in waits[:-limit]:
                        new_il.append(mk_ev(inst.engine, w))
                    si.on_wait = waits[-limit:]
                    changed = True
                new_il.append(inst)
            if changed:
                blk.instructions = new_il

    if last_blk is not None:
        il = list(last_blk.instructions)
        added = False
        for eng, h in helper_sems.items():
            for _ in range(counts[eng]):
                il.append(
                    mk_ev(
                        eng,
                        None,
                        mybir.SyncUpdate(
                            sync_type="semaphore",
                            id=h.num,
                            ant_name=h.name,
                            update_mode="sem-dec",
                            update_value=1,
                        ),
                    )
                )
                added = True
        if added:
            last_blk.instructions = il


def build_nc():
    nc = bass.Bass("TRN2", target_bir_lowering=False, debug=False)
    am_d = nc.declare_dram_parameter("am", [NM, E], BF16, isOutput=False)
    id_d = nc.declare_dram_parameter("idb", [128, 128], BF16, isOutput=False)
    mts_d = nc.declare_dram_parameter("mts", [128, KE * BS], BF16, isOutput=False)
    pwS_d = nc.declare_dram_parameter("pwS", [128, R], BF16, isOutput=False)
    idx_d = nc.declare_dram_parameter("idx", [128, R // 128], I32, isOutput=False)
    rough_d = nc.declare_dram_parameter("rough", [1, R], F32, isOutput=False)
    w1a_d = nc.declare_dram_parameter("w1a", [128, KE * HID], BF16, isOutput=False)
    w1b8_d = nc.declare_dram_parameter("w1b8", [128, NT * E], FP8, isOutput=False)
    w1c8_d = nc.declare_dram_parameter("w1c8", [128, NT * E], FP8, isOutput=False)
    w1d_d = nc.declare_dram_parameter("w1d", [PW, HID], BF16, isOutput=False)
    w2r_d = nc.declare_dram_parameter("w2r", [128, NT], BF16, isOutput=False)
    b1r_d = nc.declare_dram_parameter("b1r", [1, HID], BF16, isOutput=False)
    b2s_d = nc.declare_dram_parameter("b2s", [1, 1], F32, isOutput=False)
    out_d = nc.declare_dram_parameter("out", [1, R], F32, isOutput=True)

    helper_sems = {
        mybir.EngineType.SP: nc.alloc_semaphore("hoist_dma_sp"),
        mybir.EngineType.Pool: nc.alloc_semaphore("hoist_dma_pool"),
        mybir.EngineType.Activation: nc.alloc_semaphore("hoist_dma_act"),
    }

    with tile.TileContext(nc) as tc:
        with ExitStack() as ctx:
            const = ctx.enter_context(tc.tile_pool(name="const", bufs=1))
            w1b8_sb = const.tile([128, NT * E], FP8, tag="w1b8")
            w1c8_sb = const.tile([128, NT * E], FP8, tag="w1c8")
            wfused = const.tile([128, HID], BF16, tag="wfused")
            pwS_sb = const.tile([128, R], BF16, tag="pwS")
            mts_sb = const.tile([128, KE * BS], BF16, tag="mts")
            w2_sb = const.tile([128, NT], BF16, tag="w2")
            b1_sb = const.tile([1, HID], BF16, tag="b1")
            b2_sb = const.tile([1, 1], F32, tag="b2")
            identb = const.tile([128, 128], BF16, tag="identb")
            idx_sb = const.tile([128, R // 128], I32, tag="idx")
            ones_f = const.tile([1, BS], F32, tag="ones_f")
            ones_b = const.tile([1, BS], BF16, tag="ones_b")

            # small consts first on the SP DMA queue
            nc.sync.dma_start(identb[:], id_d[:])
            nc.sync.dma_start(b2_sb[:], b2s_d[:])
            nc.sync.dma_start(b1_sb[:], b1r_d[:])
            nc.sync.dma_start(mts_sb[:], mts_d[:])
            nc.sync.dma_start(w2_sb[:], w2r_d[:])
            nc.sync.dma_start(wfused[0:PW, :], w1d_d[:])
            nc.gpsimd.dma_start(idx_sb[:], idx_d[:])
            nc.gpsimd.memset(ones_f[:], 1.0)
            nc.vector.tensor_copy(ones_b[:], ones_f[:])

            # views
            w1b8_v = w1b8_sb[:].rearrange("p (n u c) -> p n u c", n=NT, u=KE)
            w1c8_v = w1c8_sb[:].rearrange("p (n u c) -> p n u c", n=NT, u=KE)
            mts_v = mts_sb[:].rearrange("p (e m) -> p e m", e=KE)

            gath_pool = ctx.enter_context(tc.tile_pool(name="gath", bufs=5))
            tp_pool = ctx.enter_context(tc.tile_pool(name="tp", bufs=2, space="PSUM"))
            bT_pool = ctx.enter_context(tc.tile_pool(name="bT", bufs=2))
            abT_pool = ctx.enter_context(tc.tile_pool(name="abT", bufs=2))
            h_pool = ctx.enter_context(tc.tile_pool(name="h", bufs=3))
            psH = ctx.enter_context(tc.tile_pool(name="psH", bufs=3, space="PSUM"))
            psF = ctx.enter_context(tc.tile_pool(name="psF", bufs=2, space="PSUM"))
            o_pool = ctx.enter_context(tc.tile_pool(name="o", bufs=2))
            rough_pool = ctx.enter_context(tc.tile_pool(name="rough", bufs=2))
            wa_pool = ctx.enter_context(tc.tile_pool(name="wa", bufs=3))
            taev_pool = ctx.enter_context(tc.tile_pool(name="taev", bufs=2))

            def emit_gathers(c, rc, NC):
                bT = bT_pool.tile([128, NG, 2, 512], FP8, tag="bT", name=f"bT_{c}")
                abT = abT_pool.tile([128, NG, 2, 512], FP8, tag="abT", name=f"abT_{c}")
                gs = []
                t0 = rc // 128
                for t in range(NC // 128):
                    g = gath_pool.tile([128, E], BF16, tag="g", name=f"g{c}_{t}")
                    nc.gpsimd.indirect_dma_start(
                        out=g[:], out_offset=None, in_=am_d[:],
                        in_offset=bass.IndirectOffsetOnAxis(
                            ap=idx_sb[:, t0 + t : t0 + t + 1], axis=0))
                    gs.append(g)
                return bT, abT, gs

            def transpose_unit(c, bT, abT, gs, t):
                # 8 PE transposes into one PSUM tile, then one fp8 cast
                # (Scalar) for b^T slabs + one fp8 multiply (DVE) for
                # (a*b)^T slabs.
                tp = tp_pool.tile([128, E], BF16, tag="tp", space="PSUM", name=f"tp{c}_{t}")
                for e in range(KE):
                    nc.tensor.transpose(
                        tp[:, 128 * e : 128 * (e + 1)],
                        gs[t][:, 128 * e : 128 * (e + 1)],
                        identb[:],
                    )
                sl = slice(128 * t, 128 * (t + 1))
                # out view [p, e(=2g+u), c128] over the fp8 slab tiles
                bT_o = bT[:, :, :, sl].rearrange("p g u c -> p (g u) c")
                abT_o = abT[:, :, :, sl].rearrange("p g u c -> p (g u) c")
                tp_v = tp[:].rearrange("p (e c) -> p e c", e=KE)
                nc.scalar.activation(bT_o, tp_v, mybir.ActivationFunctionType.Copy)
                nc.vector.tensor_tensor(
                    out=abT_o.rearrange("p e (t m) -> p e t m", m=BS),
                    in0=tp_v.rearrange("p e (t m) -> p e t m", m=BS),
                    in1=mts_v[:, :, None, :].to_broadcast([128, KE, 2, BS]),
                    op=mybir.AluOpType.mult)

            def phase_a(j):
                # T_a' = SC*(mentions @ W1a + b1) -> wfused[64:128, 512j:512j+512]
                ps_ta = psH.tile([128, 512], F32, tag="ps_h", name=f"ps_ta{j}")[0:BS, :]
                nc.tensor.matmul(ps_ta[:], ones_b[0:1, :], b1_sb[0:1, 512 * j : 512 * (j + 1)],
                                 start=True, stop=False)
                for k in range(KE):
                    wa_t = wa_pool.tile([128, 512], BF16, tag="wa", name=f"wa_t{j}_{k}")
                    nc.sync.dma_start(wa_t[:], w1a_d[:, HID * k + 512 * j : HID * k + 512 * (j + 1)])
                    nc.tensor.matmul(ps_ta[:], mts_v[:, k, :], wa_t[:],
                                     start=False, stop=(k == KE - 1))
                ev = taev_pool.tile([BS, 512], BF16, tag="taev", name=f"ev{j}")
                nc.vector.tensor_copy(ev[:], ps_ta[:])
                nc.sync.dma_start(wfused[PW : PW + BS, 512 * j : 512 * (j + 1)], ev[:])

            # ---- startup: PE warmup on the identity (keeps the p-state
            # ramp going while gathers land), chunk-0 transposes, phase A.
            warm_ps = ctx.enter_context(tc.tile_pool(name="wps", bufs=1, space="PSUM"))
            wps = warm_ps.tile([128, 128], BF16, tag="wps", space="PSUM")
            for _ in range(16):
                nc.tensor.transpose(wps[:], identb[:], identb[:])

            cur = emit_gathers(0, 0, _CH[0])
            for t in range(_CH[0] // 128):
                transpose_unit(0, cur[0], cur[1], cur[2], t)

            phase_a(0)
            for n in range(1):
                nc.sync.dma_start(w1b8_sb[:, E * n : E * (n + 1)], w1b8_d[:, E * n : E * (n + 1)])
                nc.sync.dma_start(w1c8_sb[:, E * n : E * (n + 1)], w1c8_d[:, E * n : E * (n + 1)])
            nc.sync.dma_start(pwS_sb[:, 0:512], pwS_d[:, 0:512])
            phase_a(1)
            for n in range(1, 3):
                nc.sync.dma_start(w1b8_sb[:, E * n : E * (n + 1)], w1b8_d[:, E * n : E * (n + 1)])
                nc.sync.dma_start(w1c8_sb[:, E * n : E * (n + 1)], w1c8_d[:, E * n : E * (n + 1)])
            nc.sync.dma_start(pwS_sb[:, 512:R], pwS_d[:, 512:R])
            for n in range(3, NT):
                nc.sync.dma_start(w1b8_sb[:, E * n : E * (n + 1)], w1b8_d[:, E * n : E * (n + 1)])
                nc.sync.dma_start(w1c8_sb[:, E * n : E * (n + 1)], w1c8_d[:, E * n : E * (n + 1)])

            # ---- main loop
            for c in range(NCHUNK):
                rc = _RCS[c]
                NC = _CH[c]
                NH = (NC + 255) // 256  # column halves of 256
                bT, abT, _ = cur
                if c + 1 < NCHUNK:
                    nxt = emit_gathers(c + 1, _RCS[c + 1], _CH[c + 1])
                    units = list(range(_CH[c + 1] // 128))
                else:
                    nxt, units = None, []
                per_group = (len(units) + NT - 1) // NT if units else 0

                ps_f = psF.tile([1, 512], F32, tag="ps_f", name=f"ps_f{c}")
                for n in range(NT):
                    nsl = slice(128 * n, 128 * (n + 1))
                    ps_h = psH.tile([128, 512], F32, tag="ps_h", name=f"ps_h{c}_{n}")
                    for h in range(NH):
                        hsl = slice(256 * h, min(256 * (h + 1), NC))
                        w = hsl.stop - hsl.start
                        for g in range(NG):
                            nc.tensor.matmul(
                                ps_h[:, hsl], w1b8_v[:, n, 2 * g : 2 * g + 2, :],
                                bT[:, g, :, hsl.start : hsl.stop],
                                start=(g == 0), stop=False, perf_mode=DRM)
                        for g in range(NG):
                            nc.tensor.matmul(
                                ps_h[:, hsl], w1c8_v[:, n, 2 * g : 2 * g + 2, :],
                                abT[:, g, :, hsl.start : hsl.stop],
                                start=False, stop=False, perf_mode=DRM)
                        nc.tensor.matmul(
                            ps_h[:, hsl], wfused[:, nsl],
                            pwS_sb[:, rc + hsl.start : rc + hsl.stop],
                            start=False, stop=True)
                    h_t = h_pool.tile([128, 512], BF16, tag="h", name=f"h{c}_{n}")
                    nc.scalar.activation(h_t[:, :NC], ps_h[:, :NC],
                                         mybir.ActivationFunctionType.Lrelu, alpha=ALPHA)
                    nc.tensor.matmul(ps_f[0:1, :NC], w2_sb[:, n : n + 1], h_t[:, :NC],
                                     start=(n == 0), stop=(n == NT - 1))
                    for _ in range(per_group):
                        if units:
                            t = units.pop(0)
                            transpose_unit(c + 1, nxt[0], nxt[1], nxt[2], t)
                rough_t = rough_pool.tile([1, 512], F32, tag="rough", name=f"ro{c}")
                nc.sync.dma_start(rough_t[0:1, :NC], rough_d[0:1, rc : rc + NC])
                o_t = o_pool.tile([1, 512], F32, tag="o", name=f"o{c}")
                nc.vector.tensor_tensor(out=o_t[0:1, :NC], in0=ps_f[0:1, :NC],
                                        in1=rough_t[0:1, :NC], op=mybir.AluOpType.add)
                nc.vector.tensor_scalar_add(o_t[0:1, :NC], o_t[0:1, :NC], b2_sb[0:1, 0:1])
                nc.sync.dma_start(out_d[0:1, rc : rc + NC], o_t[0:1, :NC])
                cur = nxt

    _redistribute_waits(nc, helper_sems)
    return nc


_NC_CACHE = None


def _get_nc():
    global _NC_CACHE
    if _NC_CACHE is None:
        _NC_CACHE = build_nc()
    return _NC_CACHE


def make_in_maps(
    all_mentions,
    mentions_batch,
    pw_batch,
    top_indices_batch,
    top_rough_scores_batch,
    W1,
    b1,
    W2,
    b2,
):
    am = np.asarray(all_mentions, np.float32)
    men = np.asarray(mentions_batch, np.float32)
    pw = np.asarray(pw_batch, np.float32)
    idx = np.asarray(top_indices_batch).astype(np.int32)
    rough = np.asarray(top_rough_scores_batch, np.float32)
    W1 = np.asarray(W1, np.float32)
    b1 = np.asarray(b1, np.float32)
    W2 = np.asarray(W2, np.float32)
    b2 = np.asarray(b2, np.float32)

    am_bf = np.ascontiguousarray(am.astype(BF_NP))

    # [p, n, kt, j] = e4m3(SC * W1x[128*kt + p, 128*n + j])
    def w8(Wx):
        w = (SC * Wx).reshape(KE, 128, NT, 128)  # [kt, p, n, j]
        w = w.transpose(1, 2, 0, 3).reshape(128, NT * E)
        return np.ascontiguousarray(w.astype(F8_NP))

    w1b8 = w8(W1[E : 2 * E])
    w1c8 = w8(W1[2 * E : 3 * E])
    # [p, kt, j] = bf16(SC * W1a[128*kt + p, j])
    w1a = (SC * W1[0:E]).reshape(KE, 128, HID).transpose(1, 0, 2).reshape(128, KE * HID)
    w1a = np.ascontiguousarray(w1a.astype(BF_NP))
    w1d = np.ascontiguousarray((SC * W1[3 * E : 3 * E + PW]).astype(BF_NP))
    w2r = np.ascontiguousarray((W2[:, 0] / SC).reshape(NT, 128).T.astype(BF_NP))
    b1r = np.ascontiguousarray((SC * b1).reshape(1, HID).astype(BF_NP))
    b2s = np.ascontiguousarray(b2.reshape(1, 1))
    S = np.tile(np.eye(BS, dtype=np.float32), (1, A))

    in_maps = []
    for c in range(NCORES):
        sl = slice(c * BS, (c + 1) * BS)
        # mts[p, e, m] = bf16(men[c*BS + m, 128*e + p])
        mts = men[sl].T.reshape(KE, 128, BS).transpose(1, 0, 2).reshape(128, KE * BS)
        mts = np.ascontiguousarray(mts.astype(BF_NP))
        pwT = pw[sl].transpose(2, 1, 0).reshape(PW, R)
        pwS = np.ascontiguousarray(np.concatenate([pwT, S], axis=0).astype(BF_NP))
        idx_r = np.ascontiguousarray(idx[sl].T.reshape(R // 128, 128).T)
        rough_r = np.ascontiguousarray(rough[sl].T.reshape(1, R))
        in_maps.append(
            dict(
                am=am_bf,
                idb=np.eye(128, dtype=BF_NP),
                mts=mts,
                pwS=pwS,
                idx=idx_r,
                rough=rough_r,
                w1a=w1a,
                w1b8=w1b8,
                w1c8=w1c8,
                w1d=w1d,
                w2r=w2r,
                b1r=b1r,
                b2s=b2s,
            )
        )
    return in_maps


def assemble_output(results):
    scores = np.empty((BATCH, A), np.float32)
    for c in range(NCORES):
        score_r = np.asarray(results[c]["out"]).reshape(A, BS)
        scores[c * BS : (c + 1) * BS, :] = score_r.T
    out = np.empty((BATCH, A + 1), np.float32)
    out[:, 0] = EPSILON
    out[:, 1:] = scores
    return out


def kernel(**inputs):
    nc = _get_nc()
    in_maps = make_in_maps(**inputs)
    res = run_bass_kernel_spmd(nc, in_maps, core_ids=list(range(NCORES)))
    return assemble_output(res.results)


if __name__ == "__main__":
    nc = build_nc()
    print("built ok")


# revision 31
# speedup vs baseline: 1.0274x; 1.0274x over previous
"""Trainium2 Bass kernel for the AnaphoricityScorer problem.

Data-parallel over the batch (mention) dimension across 8 NeuronCores.
Per core: 64 mentions x 50 antecedents = 3200 pair rows, r = ant*64 + m.

pair = [a, b, a*b, pw] @ W1 with the a-term folded into a fused k-tile
(T_a' = mentions @ W1a + b1 injected through a 0/1 selection matrix S).
The b and a*b terms run as fp8(e4m3) DoubleRow matmuls (2 k-tiles per
instruction, 2x bf16 throughput); to condition the fp8 weights, the
whole pre-activation is scaled by 64 (W1*64, b1*64 exact power-of-2
scalings) and undone through W2/64 - valid because LeakyReLU is
positively homogeneous. Gathers, transposes and the fused tile run in
bf16. Gathered rows transpose on the TensorEngine (bf16, 1 cycle/row)
into PSUM; per 128-row tile one Scalar-engine cast makes the fp8 b^T
slabs and one DVE multiply makes the fp8 (a*b)^T slabs, laid out
[128, 2, N] for DoubleRow consumption.
"""

import os
import sys
from contextlib import ExitStack

import numpy as np
import ml_dtypes

for _p in ("/opt/trn_rl_repo",):
    if _p not in sys.path and os.path.isdir(_p):
        sys.path.insert(0, _p)

from concourse import bass, mybir  # noqa: E402
import concourse.tile as tile  # noqa: E402
from concourse.masks import make_identity  # noqa: E402
from concourse.bass_utils import run_bass_kernel_spmd  # noqa: E402

NM, BATCH, A, E, PW, HID, NCORES = 2000, 512, 50, 1024, 64, 1024, 8
BS = BATCH // NCORES
R = A * BS
ALPHA, EPSILON = 0.01, 1e-07
SC = 64.0
F32 = mybir.dt.float32
BF16 = mybir.dt.bfloat16
FP8 = mybir.dt.float8e4
I32 = mybir.dt.int32
DRM = mybir.MatmulPerfMode.DoubleRow
KE, NT, NG = E // 128, HID // 128, E // 256

_CH = [256, 384, 512, 512, 512, 512, 512]
_RCS = [0, 256, 640, 1152, 1664, 2176, 2688]
NCHUNK = len(_CH)

BF_NP = ml_dtypes.bfloat16
F8_NP = ml_dtypes.float8_e4m3


def _redistribute_waits(nc, helper_sems, limit=1):
    """Enforce <=1 sync wait per instruction (walrus limit on this build).

    Compute-engine instructions execute in-order on their engine stream, so
    excess waits hoist into single-wait InstEventSemaphore instructions
    spliced just before them. DMACopy instructions execute from concurrent
    DGE queue programs, so an engine-stream EventSem does NOT gate them:
    their waits are bridged through a per-engine helper semaphore - the
    EventSems consume the original waits on the engine stream and increment
    the helper; the DMA's single wait slot watches the helper's cumulative
    count. Helpers are decremented back to zero at the end so repeated
    executions of the loaded NEFF stay correct.
    """
    counter = [0]
    counts = {e: 0 for e in helper_sems}
    last_blk = None

    def mk_ev(engine, wait=None, update=None):
        ev = mybir.InstEventSemaphore(
            name=f"hoistw-{counter[0]}", ins=[], outs=[]
        )
        counter[0] += 1
        ev.engine = engine
        ev.sync_info = mybir.SyncInfo(
            on_wait=[wait] if wait else [], on_update=[update] if update else []
        )
        return ev

    for f in nc.m.functions:
        for blk in f.blocks:
            il = blk.instructions
            if il:
                last_blk = blk
            new_il = []
            changed = False
            for inst in il:
                si = inst.sync_info
                waits = list(si.on_wait) if si is not None else []
                if isinstance(inst, mybir.InstDMACopy) and len(waits) > limit:
                    h = helper_sems[inst.engine]
                    for i, w in enumerate(waits):
                        upd = None
                        if i == len(waits) - 1:
                            upd = mybir.SyncUpdate(
                                sync_type="semaphore",
                                id=h.num,
                                ant_name=h.name,
                                update_mode="sem-inc",
                                update_value=1,
                            )
                        new_il.append(mk_ev(inst.engine, w, upd))
                    counts[inst.engine] += 1
                    si.on_wait = [
                        mybir.SyncWait(
                            sync_type="semaphore",
                            id=h.num,
                            ant_name=h.name,
                            wait_mode="sem-ge-imm",
                            wait_value=counts[inst.engine],
                        )
                    ]
                    changed = True
                elif len(waits) > limit:
                    for w in waits[:-limit]:
                        new_il.append(mk_ev(inst.engine, w))
                    si.on_wait = waits[-limit:]
                    changed = True
                new_il.append(inst)
            if changed:
                blk.instructions = new_il

    if last_blk is not None:
        il = list(last_blk.instructions)
        added = False
        for eng, h in helper_sems.items():
            for _ in range(counts[eng]):
                il.append(
                    mk_ev(
                        eng,
                        None,
                        mybir.SyncUpdate(
                            sync_type="semaphore",
                            id=h.num,
                            ant_name=h.name,
                            update_mode="sem-dec",
                            update_value=1,
                        ),
                    )
                )
                added = True
        if added:
            last_blk.instructions = il


def build_nc():
    nc = bass.Bass("TRN2", target_bir_lowering=False, debug=False)
    am_d = nc.declare_dram_parameter("am", [NM, E], BF16, isOutput=False)
    id_d = nc.declare_dram_parameter("idb", [128, 128], BF16, isOutput=False)
    mts_d = nc.declare_dram_parameter("mts", [128, KE * BS], BF16, isOutput=False)
    pwS_d = nc.declare_dram_parameter("pwS", [128, R], BF16, isOutput=False)
    idx_d = nc.declare_dram_parameter("idx", [128, R // 128], I32, isOutput=False)
    rough_d = nc.declare_dram_parameter("rough", [1, R], F32, isOutput=False)
    w1a_d = nc.declare_dram_parameter("w1a", [128, KE * HID], BF16, isOutput=False)
    w1b8_d = nc.declare_dram_parameter("w1b8", [128, NT * E], FP8, isOutput=False)
    w1c8_d = nc.declare_dram_parameter("w1c8", [128, NT * E], FP8, isOutput=False)
    w1d_d = nc.declare_dram_parameter("w1d", [PW, HID], BF16, isOutput=False)
    w2r_d = nc.declare_dram_parameter("w2r", [128, NT], BF16, isOutput=False)
    b1r_d = nc.declare_dram_parameter("b1r", [1, HID], BF16, isOutput=False)
    b2s_d = nc.declare_dram_parameter("b2s", [1, 1], F32, isOutput=False)
    out_d = nc.declare_dram_parameter("out", [1, R], F32, isOutput=True)

    helper_sems = {
        mybir.EngineType.SP: nc.alloc_semaphore("hoist_dma_sp"),
        mybir.EngineType.Pool: nc.alloc_semaphore("hoist_dma_pool"),
        mybir.EngineType.Activation: nc.alloc_semaphore("hoist_dma_act"),
    }

    with tile.TileContext(nc) as tc:
        with ExitStack() as ctx:
            const = ctx.enter_context(tc.tile_pool(name="const", bufs=1))
            w1b8_sb = const.tile([128, NT * E], FP8, tag="w1b8")
            w1c8_sb = const.tile([128, NT * E], FP8, tag="w1c8")
            wfused = const.tile([128, HID], BF16, tag="wfused")
            pwS_sb = const.tile([128, R], BF16, tag="pwS")
            mts_sb = const.tile([128, KE * BS], BF16, tag="mts")
            w2_sb = const.tile([128, NT], BF16, tag="w2")
            b1_sb = const.tile([1, HID], BF16, tag="b1")
            b2_sb = const.tile([1, 1], F32, tag="b2")
            identb = const.tile([128, 128], BF16, tag="identb")
            idx_sb = const.tile([128, R // 128], I32, tag="idx")
            rough_sb = const.tile([1, R], F32, tag="rough")
            ones_f = const.tile([1, BS], F32, tag="ones_f")
            ones_b = const.tile([1, BS], BF16, tag="ones_b")

            # small consts first on the SP DMA queue
            nc.sync.dma_start(identb[:], id_d[:])
            nc.sync.dma_start(b2_sb[:], b2s_d[:])
            nc.sync.dma_start(b1_sb[:], b1r_d[:])
            nc.sync.dma_start(mts_sb[:], mts_d[:])
            nc.sync.dma_start(w2_sb[:], w2r_d[:])
            nc.sync.dma_start(wfused[0:PW, :], w1d_d[:])
            nc.sync.dma_start(rough_sb[:], rough_d[:])
            nc.gpsimd.dma_start(idx_sb[:], idx_d[:])
            nc.gpsimd.memset(ones_f[:], 1.0)
            nc.vector.tensor_copy(ones_b[:], ones_f[:])

            # views
            w1b8_v = w1b8_sb[:].rearrange("p (n u c) -> p n u c", n=NT, u=KE)
            w1c8_v = w1c8_sb[:].rearrange("p (n u c) -> p n u c", n=NT, u=KE)
            mts_v = mts_sb[:].rearrange("p (e m) -> p e m", e=KE)

            gath_pool = ctx.enter_context(tc.tile_pool(name="gath", bufs=5))
            tp_pool = ctx.enter_context(tc.tile_pool(name="tp", bufs=2, space="PSUM"))
            bT_pool = ctx.enter_context(tc.tile_pool(name="bT", bufs=2))
            abT_pool = ctx.enter_context(tc.tile_pool(name="abT", bufs=2))
            h_pool = ctx.enter_context(tc.tile_pool(name="h", bufs=3))
            psH = ctx.enter_context(tc.tile_pool(name="psH", bufs=3, space="PSUM"))
            psF = ctx.enter_context(tc.tile_pool(name="psF", bufs=2, space="PSUM"))
            o_pool = ctx.enter_context(tc.tile_pool(name="o", bufs=2))
            wa_pool = ctx.enter_context(tc.tile_pool(name="wa", bufs=3))
            taev_pool = ctx.enter_context(tc.tile_pool(name="taev", bufs=2))

            def new_slabs(c):
                bT = bT_pool.tile([128, NG, 2, 512], FP8, tag="bT", name=f"bT_{c}")
                abT = abT_pool.tile([128, NG, 2, 512], FP8, tag="abT", name=f"abT_{c}")
                return bT, abT

            def transpose_unit(c, bT, abT, g_t, t):
                # 8 PE transposes into one PSUM tile, then one fp8 cast
                # (Scalar) for b^T slabs + one fp8 multiply (DVE) for
                # (a*b)^T slabs.
                tp = tp_pool.tile([128, E], BF16, tag="tp", space="PSUM", name=f"tp{c}_{t}")
                for e in range(KE):
                    nc.tensor.transpose(
                        tp[:, 128 * e : 128 * (e + 1)],
                        g_t[:, 128 * e : 128 * (e + 1)],
                        identb[:],
                    )
                sl = slice(128 * t, 128 * (t + 1))
                # out view [p, e(=2g+u), c128] over the fp8 slab tiles
                bT_o = bT[:, :, :, sl].rearrange("p g u c -> p (g u) c")
                abT_o = abT[:, :, :, sl].rearrange("p g u c -> p (g u) c")
                tp_v = tp[:].rearrange("p (e c) -> p e c", e=KE)
                nc.scalar.activation(bT_o, tp_v, mybir.ActivationFunctionType.Copy)
                nc.vector.tensor_tensor(
                    out=abT_o.rearrange("p e (t m) -> p e t m", m=BS),
                    in0=tp_v.rearrange("p e (t m) -> p e t m", m=BS),
                    in1=mts_v[:, :, None, :].to_broadcast([128, KE, 2, BS]),
                    op=mybir.AluOpType.mult)

            w1a_v = w1a_d[:].rearrange("p (k j) -> p k j", k=KE)

            def phase_a_dma(j):
                wa_t = wa_pool.tile([128, KE, 512], BF16, tag="wa", name=f"wa_t{j}")
                # scalar queue: parallel to the SP fp8-weight stream
                nc.scalar.dma_start(wa_t[:], w1a_v[:, :, 512 * j : 512 * (j + 1)])
                return wa_t

            def phase_a_mm(j, wa_t):
                # T_a' = SC*(mentions @ W1a + b1) -> wfused[64:128, 512j:512j+512]
                ps_ta = psH.tile([128, 512], F32, tag="ps_h", name=f"ps_ta{j}")[0:BS, :]
                nc.tensor.matmul(ps_ta[:], ones_b[0:1, :], b1_sb[0:1, 512 * j : 512 * (j + 1)],
                                 start=True, stop=False)
                for k in range(KE):
                    nc.tensor.matmul(ps_ta[:], mts_v[:, k, :], wa_t[:, k, :],
                                     start=False, stop=(k == KE - 1))
                ev = taev_pool.tile([BS, 512], BF16, tag="taev", name=f"ev{j}")
                nc.vector.tensor_copy(ev[:], ps_ta[:])
                # SP queue is drained of weight traffic by the time this waits
                nc.sync.dma_start(wfused[PW : PW + BS, 512 * j : 512 * (j + 1)], ev[:])

            # ---- startup: PE warmup on the identity (keeps the p-state
            # ramp going while gathers land), chunk-0 transposes, phase A.
            warm_ps = ctx.enter_context(tc.tile_pool(name="wps", bufs=1, space="PSUM"))
            wps = warm_ps.tile([128, 128], BF16, tag="wps", space="PSUM")
            for _ in range(16):
                nc.tensor.transpose(wps[:], identb[:], identb[:])

            # all gathers issue upfront on the Pool engine, self-throttled
            # by gath_pool buffer reuse
            gs_all = []
            for c in range(NCHUNK):
                t0 = _RCS[c] // 128
                gsc = []
                for t in range(_CH[c] // 128):
                    g = gath_pool.tile([128, E], BF16, tag="g", name=f"g{c}_{t}")
                    nc.gpsimd.indirect_dma_start(
                        out=g[:], out_offset=None, in_=am_d[:],
                        in_offset=bass.IndirectOffsetOnAxis(
                            ap=idx_sb[:, t0 + t : t0 + t + 1], axis=0))
                    gsc.append(g)
                gs_all.append(gsc)

            def w8n(n):
                nc.sync.dma_start(w1b8_sb[:, E * n : E * (n + 1)], w1b8_d[:, E * n : E * (n + 1)])
                nc.sync.dma_start(w1c8_sb[:, E * n : E * (n + 1)], w1c8_d[:, E * n : E * (n + 1)])

            wa0 = phase_a_dma(0)
            nc.scalar.dma_start(pwS_sb[:, 0:640], pwS_d[:, 0:640])
            cur = new_slabs(0)
            transpose_unit(0, cur[0], cur[1], gs_all[0][0], 0)
            w8n(0)
            phase_a_mm(0, wa0)
            for t in range(1, _CH[0] // 128):
                transpose_unit(0, cur[0], cur[1], gs_all[0][t], t)
            for n in range(1, NT):
                w8n(n)

            # ---- main loop
            for c in range(NCHUNK):
                rc = _RCS[c]
                NC = _CH[c]
                NH = (NC + 255) // 256  # column halves of 256
                bT, abT = cur
                if c + 1 < NCHUNK:
                    nxt = new_slabs(c + 1)
                    units = list(range(_CH[c + 1] // 128))
                else:
                    nxt, units = None, []
                per_group = (len(units) + NT - 1) // NT if units else 0

                ps_f = psF.tile([1, 512], F32, tag="ps_f", name=f"ps_f{c}")
                for n in range(NT):
                    if c == 0 and n == 2:
                        wa1 = phase_a_dma(1)
                        nc.scalar.dma_start(pwS_sb[:, 640:R], pwS_d[:, 640:R])
                    if c == 0 and n == 4:
                        # phase A j=1 feeds wfused cols 512:1024 (n>=4)
                        phase_a_mm(1, wa1)
                    nsl = slice(128 * n, 128 * (n + 1))
                    ps_h = psH.tile([128, 512], F32, tag="ps_h", name=f"ps_h{c}_{n}")
                    for h in range(NH):
                        hsl = slice(256 * h, min(256 * (h + 1), NC))
                        w = hsl.stop - hsl.start
                        for g in range(NG):
                            nc.tensor.matmul(
                                ps_h[:, hsl], w1b8_v[:, n, 2 * g : 2 * g + 2, :],
                                bT[:, g, :, hsl.start : hsl.stop],
                                start=(g == 0), stop=False, perf_mode=DRM)
                        for g in range(NG):
                            nc.tensor.matmul(
                                ps_h[:, hsl], w1c8_v[:, n, 2 * g : 2 * g + 2, :],
                                abT[:, g, :, hsl.start : hsl.stop],
                                start=False, stop=False, perf_mode=DRM)
                        nc.tensor.matmul(
                            ps_h[:, hsl], wfused[:, nsl],
                            pwS_sb[:, rc + hsl.start : rc + hsl.stop],
                            start=False, stop=True)
                    h_t = h_pool.tile([128, 512], BF16, tag="h", name=f"h{c}_{n}")
                    nc.scalar.activation(h_t[:, :NC], ps_h[:, :NC],
                                         mybir.ActivationFunctionType.Lrelu, alpha=ALPHA)
                    nc.tensor.matmul(ps_f[0:1, :NC], w2_sb[:, n : n + 1], h_t[:, :NC],
                                     start=(n == 0), stop=(n == NT - 1))
                    for _ in range(per_group):
                        if units:
                            t = units.pop(0)
                            transpose_unit(c + 1, nxt[0], nxt[1], gs_all[c + 1][t], t)
                o_t = o_pool.tile([1, 512], F32, tag="o", name=f"o{c}")
                nc.vector.tensor_tensor(out=o_t[0:1, :NC], in0=ps_f[0:1, :NC],
                                        in1=rough_sb[0:1, rc : rc + NC],
                                        op=mybir.AluOpType.add)
                nc.vector.tensor_scalar_add(o_t[0:1, :NC], o_t[0:1, :NC], b2_sb[0:1, 0:1])
                nc.sync.dma_start(out_d[0:1, rc : rc + NC], o_t[0:1, :NC])
                cur = nxt

    _redistribute_waits(nc, helper_sems)
    return nc


_NC_CACHE = None


def _get_nc():
    global _NC_CACHE
    if _NC_CACHE is None:
        _NC_CACHE = build_nc()
    return _NC_CACHE


def make_in_maps(
    all_mentions,
    mentions_batch,
    pw_batch,
    top_indices_batch,
    top_rough_scores_batch,
    W1,
    b1,
    W2,
    b2,
):
    am = np.asarray(all_mentions, np.float32)
    men = np.asarray(mentions_batch, np.float32)
    pw = np.asarray(pw_batch, np.float32)
    idx = np.asarray(top_indices_batch).astype(np.int32)
    rough = np.asarray(top_rough_scores_batch, np.float32)
    W1 = np.asarray(W1, np.float32)
    b1 = np.asarray(b1, np.float32)
    W2 = np.asarray(W2, np.float32)
    b2 = np.asarray(b2, np.float32)

    am_bf = np.ascontiguousarray(am.astype(BF_NP))

    # [p, n, kt, j] = e4m3(SC * W1x[128*kt + p, 128*n + j])
    def w8(Wx):
        w = (SC * Wx).reshape(KE, 128, NT, 128)  # [kt, p, n, j]
        w = w.transpose(1, 2, 0, 3).reshape(128, NT * E)
        return np.ascontiguousarray(w.astype(F8_NP))

    w1b8 = w8(W1[E : 2 * E])
    w1c8 = w8(W1[2 * E : 3 * E])
    # [p, kt, j] = bf16(SC * W1a[128*kt + p, j])
    w1a = (SC * W1[0:E]).reshape(KE, 128, HID).transpose(1, 0, 2).reshape(128, KE * HID)
    w1a = np.ascontiguousarray(w1a.astype(BF_NP))
    w1d = np.ascontiguousarray((SC * W1[3 * E : 3 * E + PW]).astype(BF_NP))
    w2r = np.ascontiguousarray((W2[:, 0] / SC).reshape(NT, 128).T.astype(BF_NP))
    b1r = np.ascontiguousarray((SC * b1).reshape(1, HID).astype(BF_NP))
    b2s = np.ascontiguousarray(b2.reshape(1, 1))
    S = np.tile(np.eye(BS, dtype=np.float32), (1, A))

    in_maps = []
    for c in range(NCORES):
        sl = slice(c * BS, (c + 1) * BS)
        # mts[p, e, m] = bf16(men[c*BS + m, 128*e + p])
        mts = men[sl].T.reshape(KE, 128, BS).transpose(1, 0, 2).reshape(128, KE * BS)
        mts = np.ascontiguousarray(mts.astype(BF_NP))
        pwT = pw[sl].transpose(2, 1, 0).reshape(PW, R)
        pwS = np.ascontiguousarray(np.concatenate([pwT, S], axis=0).astype(BF_NP))
        idx_r = np.ascontiguousarray(idx[sl].T.reshape(R // 128, 128).T)
        rough_r = np.ascontiguousarray(rough[sl].T.reshape(1, R))
        in_maps.append(
            dict(
                am=am_bf,
                idb=np.eye(128, dtype=BF_NP),
                mts=mts,
                pwS=pwS,
                idx=idx_r,
                rough=rough_r,
                w1a=w1a,
                w1b8=w1b8,
                w1c8=w1c8,
                w1d=w1d,
                w2r=w2r,
                b1r=b1r,
                b2s=b2s,
            )
        )
    return in_maps


def assemble_output(results):
    scores = np.empty((BATCH, A), np.float32)
    for c in range(NCORES):
        score_r = np.asarray(results[c]["out"]).reshape(A, BS)
        scores[c * BS : (c + 1) * BS, :] = score_r.T
    out = np.empty((BATCH, A + 1), np.float32)
    out[:, 0] = EPSILON
    out[:, 1:] = scores
    return out


def kernel(**inputs):
    nc = _get_nc()
    in_maps = make_in_maps(**inputs)
    res = run_bass_kernel_spmd(nc, in_maps, core_ids=list(range(NCORES)))
    return assemble_output(res.results)


if __name__ == "__main__":
    nc = build_nc()
    print("built ok")
